# revision 13
# baseline (speedup 1.0000x reference)
"""VQ codebook (cosine / normalized) kernel for Trainium2, 8 NeuronCores SPMD.

Reference computation (see problem):
  zf = transpose(z, bchw->bhwc).reshape(N, 64); zfn = l2norm(zf)
  wn = l2norm(weight)                    # [8192, 64]
  d = zfn @ wn.T; idx = argmax(d, 1)     # [N]
  z_q = wn[idx]  (straight-through => z_q_out == z_q numerically)
  loss = 1.01 * mean((z_q - zfn)**2)     # = 1.01 * mean_n(2 - 2*cos_n)/64
  encodings = one_hot(idx); perplexity from avg_probs

Device (per core, data-parallel over tokens, codebook replicated):
  - normalize codebook rows (batched); build whT/wlT fp16 hi/lo parts
    (x256 scale) via DRAM round-trip + DMA transpose
  - scores = zf_shard @ wn.T computed EXACTLY as 3 fp16 matmuls:
    zh*wh + zl*wh + zh*wl (error ~1e-6 abs, 10x below min top-2 gap;
    un-normalized zf: argmax-invariant)
  - argmax over 8192 per token: one column-wise max-reduce pass
    (acc[j] = max_c S[c*512+j]) -> j0 via max_index; chunk recovered with
    a gpsimd wrapped-index gather of S[:, 512c + j0] for all c
  - per-token loss term e = 2 - 2*smax/(65536*|zf|)
Host: gather shards, build one-hot / z_q / scalars from device indices.
"""

import sys

sys.path.insert(0, "/opt/trn_rl_repo")

import numpy as np

import concourse.bass as bass
import concourse.mybir as mybir
from concourse.tile import TileContext

F32 = mybir.dt.float32
F16 = mybir.dt.float16
I32 = mybir.dt.int32
U32 = mybir.dt.uint32
U16 = mybir.dt.uint16

N_CORES = 8
B, C, H, W = 16, 64, 32, 32
N = B * H * W  # 16384 tokens
D = 64
K = 8192
TPC = N // N_CORES  # 2048 tokens per core
P = 128
NT = TPC // P  # 16 token tiles per core
CHW = 512  # score chunk width (one PSUM bank)
NCH = K // CHW  # 16 chunks
QW = 2048  # psum quarter width (4 banks)
NQ = K // QW  # 4 quarters
SCALE = 256.0
INV_SCALE2 = 1.0 / 65536.0

# token tiles whose column-max pass runs on gpsimd instead of DVE
GP_TILES = set()

_CACHE = {}

# This walrus encodes at most ONE sync wait per instruction.
_CTRL_OPCODES = {"Drain", "NoOp", "EventSemaphore", "AllEngineBarrier", "Halt"}


def split_waits(nc, max_compute=1):
    """Split excess sem waits onto preceding InstNoOp instructions on the
    same engine (engine waits execute in order: semantics-preserving)."""
    n_new = 0
    for f in nc.m.functions:
        for bb in f.blocks:
            insts = bb.instructions
            i = 0
            while i < len(insts):
                inst = insts[i]
                si = inst.sync_info
                waits = list(si.on_wait) if (si and si.on_wait) else []
                cap = 1 if str(inst.opcode) in _CTRL_OPCODES else max_compute
                if len(waits) > cap:
                    keep = waits[-cap:]
                    extra = waits[:-cap]
                    pos = i
                    for j in range(len(extra)):
                        nop = mybir.InstNoOp(name=f"{inst.name}-wnop{j}")
                        nop.engine = inst.engine
                        nop.sync_info = mybir.SyncInfo(
                            on_wait=[extra[j]], on_update=[]
                        )
                        insts.insert(pos, nop)
                        pos += 1
                        n_new += 1
                    si.on_wait = keep
                    i = pos
                i += 1
    return n_new


def build_nc(split=True):
    nc = bass.Bass()

    zt_d = nc.declare_dram_parameter("zt", [D, TPC], F32, isOutput=False)
    ztok_d = nc.declare_dram_parameter("ztok", [TPC, D], F32, isOutput=False)
    w_d = nc.declare_dram_parameter("weight", [K, D], F32, isOutput=False)
    idx_d = nc.declare_dram_parameter("idx_out", [P, NT], I32, isOutput=True)
    loss_d = nc.declare_dram_parameter("loss_e", [P, NT], F32, isOutput=True)
    wn_d = nc.declare_dram_parameter("wn_out", [K, D], F32, isOutput=True)

    mult = mybir.AluOpType.mult
    add = mybir.AluOpType.add
    sub = mybir.AluOpType.subtract
    shr = mybir.AluOpType.logical_shift_right
    shl = mybir.AluOpType.logical_shift_left
    COPY = mybir.ActivationFunctionType.Copy
    X = mybir.AxisListType.X

    with TileContext(nc) as tc:
        with (
            tc.tile_pool(name="const", bufs=1) as const,
            tc.tile_pool(name="big", bufs=1) as big,
            tc.tile_pool(name="wp", bufs=1) as wp,
            tc.tile_pool(name="scores", bufs=2) as spool,
            tc.tile_pool(name="small", bufs=3) as small,
            tc.tile_pool(name="dram", bufs=1, space="DRAM") as dram,
            tc.tile_pool(name="mm", bufs=2, space="PSUM") as psum,
        ):
            # ---- constants ----
            iota16 = const.tile([P, NCH], U16, tag="iota16")
            nc.gpsimd.iota(iota16[:], pattern=[[CHW, NCH]], base=0,
                           channel_multiplier=0)

            # ---- persistent tiles ----
            whT = big.tile([D, K], F16, tag="whT")
            wlT = big.tile([D, K], F16, tag="wlT")
            zh = big.tile([D, TPC], F16, tag="zh")
            zl = big.tile([D, TPC], F16, tag="zl")
            idx_all = big.tile([P, NT], I32, tag="idx")
            loss_all = big.tile([P, NT], F32, tag="loss")
            zri_all = big.tile([P, NT], F32, tag="zri")

            # ---- z prep ----
            zt_sb = wp.tile([D, TPC], F32, tag="ztmp")
            nc.sync.dma_start(zt_sb[:], zt_d[:, :])
            nc.scalar.activation(zh[:], zt_sb[:], COPY, scale=SCALE)
            nc.vector.scalar_tensor_tensor(zl[:], zt_sb[:], SCALE, zh[:],
                                           op0=mult, op1=sub)

            # token norms, batched: ztok rows (t*128+p) -> [p, t, d]
            ztok_b = wp.tile([P, NT * D], F32, tag="ztmp")
            nc.sync.dma_start(
                ztok_b[:].rearrange("p (t d) -> p t d", d=D),
                ztok_d.rearrange("(t p) d -> p t d", p=P),
            )
            zsq = wp.tile([P, NT * D], F32, tag="scratch16")
            nc.vector.tensor_mul(zsq[:], ztok_b[:], ztok_b[:])
            zn2 = small.tile([P, NT], F32, tag="zn2")
            nc.vector.reduce_sum(
                zn2[:], zsq[:].rearrange("p (t d) -> p t d", d=D), axis=X
            )
            znr = small.tile([P, NT], F32, tag="znr")
            nc.scalar.sqrt(znr[:], zn2[:])
            nc.vector.reciprocal(zri_all[:], znr[:])

            # ---- codebook prep (batched): rows (p*64+t) -> [p, t, d] ----
            w_big = wp.tile([P, 64 * D], F32, tag="wbig")
            nc.sync.dma_start(
                w_big[:], w_d.rearrange("(p t) d -> p (t d)", p=P)
            )
            wsq = wp.tile([P, 64 * D], F32, tag="scratch16")
            nc.vector.tensor_mul(wsq[:], w_big[:], w_big[:])
            wn2 = small.tile([P, 64], F32, tag="wn2")
            nc.vector.reduce_sum(
                wn2[:], wsq[:].rearrange("p (t d) -> p t d", d=D), axis=X
            )
            wnr = small.tile([P, 64], F32, tag="wnr")
            nc.scalar.sqrt(wnr[:], wn2[:])
            wri = small.tile([P, 64], F32, tag="wri")
            nc.vector.reciprocal(wri[:], wnr[:])
            nc.vector.tensor_mul(
                w_big[:].rearrange("p (t d) -> p t d", d=D),
                w_big[:].rearrange("p (t d) -> p t d", d=D),
                wri[:].to_broadcast([P, 64, D]),
            )
            nc.sync.dma_start(
                wn_d.rearrange("(p t) d -> p (t d)", p=P), w_big[:]
            )
            wh_big = wp.tile([P, 64 * D], F16, tag="whbig")
            nc.scalar.activation(wh_big[:], w_big[:], COPY, scale=SCALE)
            wl_big = wp.tile([P, 64 * D], F16, tag="wlbig")
            nc.vector.scalar_tensor_tensor(wl_big[:], w_big[:], SCALE,
                                           wh_big[:], op0=mult, op1=sub)
            wh_rm = dram.tile([K, D], F16, tag="whrm")
            wl_rm = dram.tile([K, D], F16, tag="wlrm")
            nc.sync.dma_start(
                wh_rm[:].rearrange("(p t) d -> p (t d)", p=P), wh_big[:]
            )
            nc.sync.dma_start(
                wl_rm[:].rearrange("(p t) d -> p (t d)", p=P), wl_big[:]
            )
            nc.sync.dma_start_transpose(whT[:], wh_rm[:])
            nc.sync.dma_start_transpose(wlT[:], wl_rm[:])

            # ---- per token tile ----
            for t in range(NT):
                scores = spool.tile([P, K], F32, tag="scores")
                zh_t = zh[:, t * P : (t + 1) * P]
                zl_t = zl[:, t * P : (t + 1) * P]
                for q in range(NQ):
                    ps = psum.tile([P, QW], F32, tag="mm")
                    for cc in range(QW // CHW):
                        c = q * (QW // CHW) + cc
                        sl = ps[:, cc * CHW : (cc + 1) * CHW]
                        wsl = slice(c * CHW, (c + 1) * CHW)
                        nc.tensor.matmul(sl, lhsT=zh_t, rhs=whT[:, wsl],
                                         start=True, stop=False)
                        nc.tensor.matmul(sl, lhsT=zl_t, rhs=whT[:, wsl],
                                         start=False, stop=False)
                        nc.tensor.matmul(sl, lhsT=zh_t, rhs=wlT[:, wsl],
                                         start=False, stop=True)
                    nc.scalar.copy(scores[:, q * QW : (q + 1) * QW], ps[:])

                # column-wise max over the 16 chunks: acc[j] = max_c S[c*512+j]
                acc = small.tile([P, CHW], F32, tag="acc")
                if t in GP_TILES:
                    g1 = wp.tile([P, 4096], F32, tag="scratch16")
                    nc.gpsimd.tensor_max(g1[:], scores[:, :4096],
                                         scores[:, 4096:])
                    nc.vector.reduce_max(
                        acc[:],
                        g1[:].rearrange("p (c j) -> p j c", j=CHW),
                        axis=X,
                    )
                else:
                    nc.vector.reduce_max(
                        acc[:],
                        scores[:].rearrange("p (c j) -> p j c", j=CHW),
                        axis=X,
                    )
                gmax = small.tile([P, 1], F32, tag="gmax")
                nc.vector.reduce_max(gmax[:], acc[:], axis=X)
                in8 = small.tile([P, 8], F32, tag="in8")
                nc.gpsimd.memset(in8[:], -3.0e38)
                nc.vector.tensor_copy(in8[:, 0:1], gmax[:])
                j08 = small.tile([P, 8], U32, tag="j08")
                nc.vector.max_index(j08[:], in8[:], acc[:])
                # chunk recovery: gather S[:, 512c + j0] for all c via the
                # group-wrapped gpsimd gather (idxs[p, c] = 512c + j0[p])
                j0f = small.tile([P, 1], F32, tag="j0f")
                nc.vector.tensor_copy(j0f[:], j08[:, 0:1])
                gidx = small.tile([P, NCH], U16, tag="gidx")
                nc.vector.tensor_scalar(
                    out=gidx[:], in0=iota16[:], scalar1=j0f[:],
                    scalar2=None, op0=add,
                )
                g256 = small.tile([P, P * 2], F32, tag="g256")
                nc.gpsimd.indirect_copy(g256[:], scores[:], gidx[:], True)
                k8 = small.tile([P, 8], U32, tag="k8")
                nc.vector.max_index(k8[:], in8[:], g256[:])
                cidx = small.tile([P, 1], U32, tag="cidx")
                nc.vector.tensor_scalar(
                    out=cidx[:], in0=k8[:, 0:1], scalar1=4, scalar2=9,
                    op0=shr, op1=shl,
                )
                idxf = small.tile([P, 1], U32, tag="idxf")
                nc.vector.tensor_add(idxf[:], cidx[:], j08[:, 0:1])
                nc.vector.tensor_copy(idx_all[:, t : t + 1], idxf[:])

                # loss term: e = 2 - (2/65536) * gmax * (1/|zf|)
                cosv = small.tile([P, 1], F32, tag="cosv")
                nc.vector.tensor_mul(cosv[:], gmax[:],
                                     zri_all[:, t : t + 1])
                nc.vector.tensor_scalar(
                    out=loss_all[:, t : t + 1], in0=cosv[:],
                    scalar1=-2.0 * INV_SCALE2, scalar2=2.0,
                    op0=mult, op1=add,
                )

            nc.sync.dma_start(idx_d[:, :], idx_all[:])
            nc.sync.dma_start(loss_d[:, :], loss_all[:])

    if split:
        split_waits(nc)
    return nc


def make_in_maps(z, weight):
    z = np.ascontiguousarray(z, dtype=np.float32)
    w = np.ascontiguousarray(weight, dtype=np.float32)
    zf = np.ascontiguousarray(z.transpose(0, 2, 3, 1).reshape(N, D))
    in_maps = []
    for c in range(N_CORES):
        sh = zf[c * TPC : (c + 1) * TPC]
        in_maps.append(
            {
                "zt": np.ascontiguousarray(sh.T),
                "ztok": np.ascontiguousarray(sh),
                "weight": w,
            }
        )
    return in_maps


def assemble(results):
    idx_parts = []
    loss_sum = 0.0
    for r in results:
        idx_parts.append(np.ascontiguousarray(r["idx_out"].T).reshape(-1))
        loss_sum += r["loss_e"].astype(np.float64).sum()
    idx = np.concatenate(idx_parts).astype(np.int32)
    wn = np.asarray(results[0]["wn_out"], dtype=np.float32)

    z_q_flat = wn[idx]
    z_q_out = np.ascontiguousarray(
        z_q_flat.reshape(B, H, W, C).transpose(0, 3, 1, 2)
    ).astype(np.float32)

    encodings = np.zeros((N, K), dtype=np.float32)
    encodings[np.arange(N), idx] = 1.0

    loss = np.float32(1.01 * loss_sum / (N * D))

    counts = np.bincount(idx, minlength=K).astype(np.float64)
    avg_probs = counts / N
    perplexity = np.float32(np.exp(-np.sum(avg_probs * np.log(avg_probs + 1e-10))))

    return z_q_out, idx, loss, encodings, perplexity


def kernel(z, weight):
    from concourse.bass_utils import run_bass_kernel_spmd

    if "nc" not in _CACHE:
        _CACHE["nc"] = build_nc()
    nc = _CACHE["nc"]
    in_maps = make_in_maps(z, weight)
    res = run_bass_kernel_spmd(nc, in_maps, list(range(N_CORES)))
    return assemble(res.results)


# revision 20
# speedup vs baseline: 3.1609x; 3.1609x over previous
"""VQ codebook (cosine / normalized) kernel for Trainium2, 8 NeuronCores SPMD.

Reference computation (see problem):
  zf = transpose(z, bchw->bhwc).reshape(N, 64); zfn = l2norm(zf)
  wn = l2norm(weight)                    # [8192, 64]
  d = zfn @ wn.T; idx = argmax(d, 1)     # [N]
  z_q = wn[idx]  (straight-through => z_q_out == z_q numerically)
  loss = 1.01 * mean((z_q - zfn)**2)     # = 1.01 * mean_n(2 - 2*cos_n)/64
  encodings = one_hot(idx); perplexity from avg_probs

Device (per core, data-parallel over tokens, codebook replicated):
  - normalize codebook rows (batched); build whT/wlT fp16 hi/lo parts
    (x256 scale) via DRAM round-trip + DMA transpose
  - scores = zf_shard @ wn.T computed EXACTLY as 3 fp16 matmuls:
    zh*wh + zl*wh + zh*wl (error ~1e-6 abs, 10x below min top-2 gap;
    un-normalized zf: argmax-invariant)
  - argmax over 8192 per token: one column-wise max-reduce pass
    (acc[j] = max_c S[c*512+j]) -> j0 via max_index; chunk recovered with
    a gpsimd wrapped-index gather of S[:, 512c + j0] for all c
  - per-token loss term e = 2 - 2*smax/(65536*|zf|)
Host: gather shards, build one-hot / z_q / scalars from device indices.
"""

import sys

sys.path.insert(0, "/opt/trn_rl_repo")

import numpy as np

import concourse.bass as bass
import concourse.mybir as mybir
from concourse.masks import make_identity
from concourse.tile import TileContext

F32 = mybir.dt.float32
F16 = mybir.dt.float16
I32 = mybir.dt.int32
U32 = mybir.dt.uint32
U16 = mybir.dt.uint16

N_CORES = 8
B, C, H, W = 16, 64, 32, 32
N = B * H * W  # 16384 tokens
D = 64
K = 8192
TPC = N // N_CORES  # 2048 tokens per core
P = 128
NT = TPC // P  # 16 token tiles per core
CHW = 512  # score chunk width (one PSUM bank)
NCH = K // CHW  # 16 chunks
QW = 2048  # psum quarter width (4 banks)
NQ = K // QW  # 4 quarters
SCALE = 256.0
INV_SCALE2 = 1.0 / 65536.0

# token tiles whose column-max pass runs on gpsimd instead of DVE
GP_TILES = set()

_CACHE = {}

# This walrus encodes at most ONE sync wait per instruction.
_CTRL_OPCODES = {"Drain", "NoOp", "EventSemaphore", "AllEngineBarrier", "Halt"}


def split_waits(nc, max_compute=1):
    """Split excess sem waits onto preceding InstNoOp instructions on the
    same engine (engine waits execute in order: semantics-preserving)."""
    n_new = 0
    for f in nc.m.functions:
        for bb in f.blocks:
            insts = bb.instructions
            i = 0
            while i < len(insts):
                inst = insts[i]
                si = inst.sync_info
                waits = list(si.on_wait) if (si and si.on_wait) else []
                cap = 1 if str(inst.opcode) in _CTRL_OPCODES else max_compute
                if len(waits) > cap:
                    keep = waits[-cap:]
                    extra = waits[:-cap]
                    pos = i
                    for j in range(len(extra)):
                        nop = mybir.InstNoOp(name=f"{inst.name}-wnop{j}")
                        nop.engine = inst.engine
                        nop.sync_info = mybir.SyncInfo(
                            on_wait=[extra[j]], on_update=[]
                        )
                        insts.insert(pos, nop)
                        pos += 1
                        n_new += 1
                    si.on_wait = keep
                    i = pos
                i += 1
    return n_new


def build_nc(split=True):
    nc = bass.Bass()

    zt_d = nc.declare_dram_parameter("zt", [D, TPC], F32, isOutput=False)
    ztok_d = nc.declare_dram_parameter("ztok", [TPC, D], F32, isOutput=False)
    w_d = nc.declare_dram_parameter("weight", [K, D], F32, isOutput=False)
    idx_d = nc.declare_dram_parameter("idx_out", [P, NT], I32, isOutput=True)
    loss_d = nc.declare_dram_parameter("loss_e", [P, NT], F32, isOutput=True)
    wn_d = nc.declare_dram_parameter("wn_out", [K, D], F32, isOutput=True)

    mult = mybir.AluOpType.mult
    add = mybir.AluOpType.add
    sub = mybir.AluOpType.subtract
    shr = mybir.AluOpType.logical_shift_right
    shl = mybir.AluOpType.logical_shift_left
    COPY = mybir.ActivationFunctionType.Copy
    X = mybir.AxisListType.X

    with TileContext(nc) as tc:
        with (
            tc.tile_pool(name="const", bufs=1) as const,
            tc.tile_pool(name="big", bufs=1) as big,
            tc.tile_pool(name="wp", bufs=1) as wp,
            tc.tile_pool(name="scores", bufs=2) as spool,
            tc.tile_pool(name="small", bufs=3) as small,
            tc.tile_pool(name="dram", bufs=1, space="DRAM") as dram,
            tc.tile_pool(name="mm", bufs=2, space="PSUM") as psum,
        ):
            # ---- constants ----
            iota16 = const.tile([P, NCH], U16, tag="iota16")
            nc.gpsimd.iota(iota16[:], pattern=[[CHW, NCH]], base=0,
                           channel_multiplier=0)
            ident = const.tile([P, P], F16, tag="ident")
            make_identity(nc, ident[:])

            # ---- persistent tiles ----
            whlT = big.tile([D, 2 * K], F16, tag="whlT")
            whT = whlT[:, :K]
            wlT = whlT[:, K:]
            zh = big.tile([D, TPC], F16, tag="zh")
            zl = big.tile([D, TPC], F16, tag="zl")
            idx_all = big.tile([P, NT], I32, tag="idx")
            loss_all = big.tile([P, NT], F32, tag="loss")
            zri_all = big.tile([P, NT], F32, tag="zri")

            # ---- z prep ----
            zt_sb = wp.tile([D, TPC], F32, tag="ztmp")
            nc.sync.dma_start(zt_sb[:], zt_d[:, :])
            nc.scalar.activation(zh[:], zt_sb[:], COPY, scale=SCALE)
            nc.vector.scalar_tensor_tensor(zl[:], zt_sb[:], SCALE, zh[:],
                                           op0=mult, op1=sub)

            # token norms, batched: ztok rows (t*128+p) -> [p, t, d]
            ztok_b = wp.tile([P, NT * D], F32, tag="ztmp")
            nc.sync.dma_start(
                ztok_b[:].rearrange("p (t d) -> p t d", d=D),
                ztok_d.rearrange("(t p) d -> p t d", p=P),
            )
            zsq = wp.tile([P, NT * D], F32, tag="scratch16")
            nc.vector.tensor_mul(zsq[:], ztok_b[:], ztok_b[:])
            zn2 = small.tile([P, NT], F32, tag="zn2")
            nc.vector.reduce_sum(
                zn2[:], zsq[:].rearrange("p (t d) -> p t d", d=D), axis=X
            )
            znr = small.tile([P, NT], F32, tag="znr")
            nc.scalar.sqrt(znr[:], zn2[:])
            nc.vector.reciprocal(zri_all[:], znr[:])

            # ---- codebook prep (batched): rows (p*64+t) -> [p, t, d] ----
            w_big = wp.tile([P, 64 * D], F32, tag="wbig")
            nc.sync.dma_start(
                w_big[:], w_d.rearrange("(p t) d -> p (t d)", p=P)
            )
            wsq = wp.tile([P, 64 * D], F32, tag="scratch16")
            nc.vector.tensor_mul(wsq[:], w_big[:], w_big[:])
            wn2 = small.tile([P, 64], F32, tag="wn2")
            nc.vector.reduce_sum(
                wn2[:], wsq[:].rearrange("p (t d) -> p t d", d=D), axis=X
            )
            wnr = small.tile([P, 64], F32, tag="wnr")
            nc.scalar.sqrt(wnr[:], wn2[:])
            wri = small.tile([P, 64], F32, tag="wri")
            nc.vector.reciprocal(wri[:], wnr[:])
            nc.vector.tensor_mul(
                w_big[:].rearrange("p (t d) -> p t d", d=D),
                w_big[:].rearrange("p (t d) -> p t d", d=D),
                wri[:].to_broadcast([P, 64, D]),
            )
            nc.sync.dma_start(
                wn_d.rearrange("(p t) d -> p (t d)", p=P), w_big[:]
            )
            wh_big = wp.tile([P, 64 * D], F16, tag="whbig")
            nc.scalar.activation(wh_big[:], w_big[:], COPY, scale=SCALE)
            wl_big = wp.tile([P, 64 * D], F16, tag="wlbig")
            nc.vector.scalar_tensor_tensor(wl_big[:], w_big[:], SCALE,
                                           wh_big[:], op0=mult, op1=sub)
            # PE-transpose wh/wl into whT/wlT.
            # wh_big[p, t*64+d] holds code row p*64+t; transposing the
            # [128, 64] slice for tile t yields psum[d, p] = code p*64+t,
            # scattered into whT columns t + 64*p (stride-64 writes keep
            # whT column c == code id c).
            for part, src, dstT in ((0, wh_big, whT), (1, wl_big, wlT)):
                for b in range(4):  # 16 transposes per psum tile
                    pt = psum.tile([D, 16 * P], F16, tag="mm")
                    for j in range(16):
                        t_i = b * 16 + j
                        nc.tensor.transpose(
                            pt[:, j * P : (j + 1) * P],
                            src[:, t_i * D : (t_i + 1) * D],
                            ident[:],
                        )
                    eng = nc.scalar.copy if part == 0 else nc.vector.tensor_copy
                    eng(
                        dstT.rearrange("e (p t) -> e t p", t=64)[
                            :, b * 16 : (b + 1) * 16, :
                        ],
                        pt[:, :].rearrange("e (j p) -> e j p", p=P),
                    )

            # ---- per token tile ----
            for t in range(NT):
                scores = spool.tile([P, K], F32, tag="scores")
                zh_t = zh[:, t * P : (t + 1) * P]
                zl_t = zl[:, t * P : (t + 1) * P]
                for q in range(NQ):
                    ps = psum.tile([P, QW], F32, tag="mm")
                    for pi, (lt, wt, st, sp) in enumerate(
                        ((zh_t, whT, True, False), (zl_t, whT, False, False),
                         (zh_t, wlT, False, True))
                    ):
                        for cc in range(QW // CHW):
                            c = q * (QW // CHW) + cc
                            sl = ps[:, cc * CHW : (cc + 1) * CHW]
                            wsl = slice(c * CHW, (c + 1) * CHW)
                            nc.tensor.matmul(sl, lhsT=lt, rhs=wt[:, wsl],
                                             start=st, stop=sp)
                    nc.scalar.copy(scores[:, q * QW : (q + 1) * QW], ps[:])

                # column-wise max over the 16 chunks: acc[j] = max_c S[c*512+j]
                acc = small.tile([P, CHW], F32, tag="acc")
                if t in GP_TILES:
                    g1 = wp.tile([P, 4096], F32, tag="scratch16")
                    nc.gpsimd.tensor_max(g1[:], scores[:, :4096],
                                         scores[:, 4096:])
                    nc.vector.reduce_max(
                        acc[:],
                        g1[:].rearrange("p (c j) -> p j c", j=CHW),
                        axis=X,
                    )
                else:
                    nc.vector.reduce_max(
                        acc[:],
                        scores[:].rearrange("p (c j) -> p j c", j=CHW),
                        axis=X,
                    )
                gmax = small.tile([P, 1], F32, tag="gmax")
                nc.vector.reduce_max(gmax[:], acc[:], axis=X)
                in8 = small.tile([P, 8], F32, tag="in8")
                nc.gpsimd.memset(in8[:], -3.0e38)
                nc.vector.tensor_copy(in8[:, 0:1], gmax[:])
                j08 = small.tile([P, 8], U32, tag="j08")
                nc.vector.max_index(j08[:], in8[:], acc[:])
                # chunk recovery: gather S[:, 512c + j0] for all c via the
                # group-wrapped gpsimd gather (idxs[p, c] = 512c + j0[p])
                j0f = small.tile([P, 1], F32, tag="j0f")
                nc.vector.tensor_copy(j0f[:], j08[:, 0:1])
                gidx = small.tile([P, NCH], U16, tag="gidx")
                nc.vector.tensor_scalar(
                    out=gidx[:], in0=iota16[:], scalar1=j0f[:],
                    scalar2=None, op0=add,
                )
                g256 = small.tile([P, P * 2], F32, tag="g256")
                nc.gpsimd.indirect_copy(g256[:], scores[:], gidx[:], True)
                k8 = small.tile([P, 8], U32, tag="k8")
                nc.vector.max_index(k8[:], in8[:], g256[:])
                cidx = small.tile([P, 1], U32, tag="cidx")
                nc.vector.tensor_scalar(
                    out=cidx[:], in0=k8[:, 0:1], scalar1=4, scalar2=9,
                    op0=shr, op1=shl,
                )
                idxf = small.tile([P, 1], U32, tag="idxf")
                nc.vector.tensor_add(idxf[:], cidx[:], j08[:, 0:1])
                nc.vector.tensor_copy(idx_all[:, t : t + 1], idxf[:])

                # loss term: e = 2 - (2/65536) * gmax * (1/|zf|)
                cosv = small.tile([P, 1], F32, tag="cosv")
                nc.vector.tensor_mul(cosv[:], gmax[:],
                                     zri_all[:, t : t + 1])
                nc.vector.tensor_scalar(
                    out=loss_all[:, t : t + 1], in0=cosv[:],
                    scalar1=-2.0 * INV_SCALE2, scalar2=2.0,
                    op0=mult, op1=add,
                )

            nc.sync.dma_start(idx_d[:, :], idx_all[:])
            nc.sync.dma_start(loss_d[:, :], loss_all[:])

    if split:
        split_waits(nc)
    return nc


def make_in_maps(z, weight):
    z = np.ascontiguousarray(z, dtype=np.float32)
    w = np.ascontiguousarray(weight, dtype=np.float32)
    zf = np.ascontiguousarray(z.transpose(0, 2, 3, 1).reshape(N, D))
    in_maps = []
    for c in range(N_CORES):
        sh = zf[c * TPC : (c + 1) * TPC]
        in_maps.append(
            {
                "zt": np.ascontiguousarray(sh.T),
                "ztok": np.ascontiguousarray(sh),
                "weight": w,
            }
        )
    return in_maps


def assemble(results):
    idx_parts = []
    loss_sum = 0.0
    for r in results:
        idx_parts.append(np.ascontiguousarray(r["idx_out"].T).reshape(-1))
        loss_sum += r["loss_e"].astype(np.float64).sum()
    idx = np.concatenate(idx_parts).astype(np.int32)
    wn = np.asarray(results[0]["wn_out"], dtype=np.float32)

    z_q_flat = wn[idx]
    z_q_out = np.ascontiguousarray(
        z_q_flat.reshape(B, H, W, C).transpose(0, 3, 1, 2)
    ).astype(np.float32)

    encodings = np.zeros((N, K), dtype=np.float32)
    encodings[np.arange(N), idx] = 1.0

    loss = np.float32(1.01 * loss_sum / (N * D))

    counts = np.bincount(idx, minlength=K).astype(np.float64)
    avg_probs = counts / N
    perplexity = np.float32(np.exp(-np.sum(avg_probs * np.log(avg_probs + 1e-10))))

    return z_q_out, idx, loss, encodings, perplexity


def kernel(z, weight):
    from concourse.bass_utils import run_bass_kernel_spmd

    if "nc" not in _CACHE:
        _CACHE["nc"] = build_nc()
    nc = _CACHE["nc"]
    in_maps = make_in_maps(z, weight)
    res = run_bass_kernel_spmd(nc, in_maps, list(range(N_CORES)))
    return assemble(res.results)


# revision 21
# speedup vs baseline: 3.4468x; 1.0905x over previous
"""VQ codebook (cosine / normalized) kernel for Trainium2, 8 NeuronCores SPMD.

Reference computation (see problem):
  zf = transpose(z, bchw->bhwc).reshape(N, 64); zfn = l2norm(zf)
  wn = l2norm(weight)                    # [8192, 64]
  d = zfn @ wn.T; idx = argmax(d, 1)     # [N]
  z_q = wn[idx]  (straight-through => z_q_out == z_q numerically)
  loss = 1.01 * mean((z_q - zfn)**2)     # = 1.01 * mean_n(2 - 2*cos_n)/64
  encodings = one_hot(idx); perplexity from avg_probs

Device (per core, data-parallel over tokens, codebook replicated):
  - normalize codebook rows (batched); build whT/wlT fp16 hi/lo parts
    (x256 scale) via DRAM round-trip + DMA transpose
  - scores = zf_shard @ wn.T computed EXACTLY as 3 fp16 matmuls:
    zh*wh + zl*wh + zh*wl (error ~1e-6 abs, 10x below min top-2 gap;
    un-normalized zf: argmax-invariant)
  - argmax over 8192 per token: one column-wise max-reduce pass
    (acc[j] = max_c S[c*512+j]) -> j0 via max_index; chunk recovered with
    a gpsimd wrapped-index gather of S[:, 512c + j0] for all c
  - per-token loss term e = 2 - 2*smax/(65536*|zf|)
Host: gather shards, build one-hot / z_q / scalars from device indices.
"""

import sys

sys.path.insert(0, "/opt/trn_rl_repo")

import numpy as np

import concourse.bass as bass
import concourse.mybir as mybir
from concourse.masks import make_identity
from concourse.tile import TileContext

F32 = mybir.dt.float32
F16 = mybir.dt.float16
I32 = mybir.dt.int32
U32 = mybir.dt.uint32
U16 = mybir.dt.uint16

N_CORES = 8
B, C, H, W = 16, 64, 32, 32
N = B * H * W  # 16384 tokens
D = 64
K = 8192
TPC = N // N_CORES  # 2048 tokens per core
P = 128
NT = TPC // P  # 16 token tiles per core
CHW = 512  # score chunk width (one PSUM bank)
NCH = K // CHW  # 16 chunks
QW = 2048  # psum quarter width (4 banks)
NQ = K // QW  # 4 quarters
SCALE = 256.0
INV_SCALE2 = 1.0 / 65536.0

# token tiles whose column-max pass runs on gpsimd instead of DVE
GP_TILES = set()

_CACHE = {}

# This walrus encodes at most ONE sync wait per instruction.
_CTRL_OPCODES = {"Drain", "NoOp", "EventSemaphore", "AllEngineBarrier", "Halt"}


def split_waits(nc, max_compute=1):
    """Split excess sem waits onto preceding InstNoOp instructions on the
    same engine (engine waits execute in order: semantics-preserving)."""
    n_new = 0
    for f in nc.m.functions:
        for bb in f.blocks:
            insts = bb.instructions
            i = 0
            while i < len(insts):
                inst = insts[i]
                si = inst.sync_info
                waits = list(si.on_wait) if (si and si.on_wait) else []
                cap = 1 if str(inst.opcode) in _CTRL_OPCODES else max_compute
                if len(waits) > cap:
                    keep = waits[-cap:]
                    extra = waits[:-cap]
                    pos = i
                    for j in range(len(extra)):
                        nop = mybir.InstNoOp(name=f"{inst.name}-wnop{j}")
                        nop.engine = inst.engine
                        nop.sync_info = mybir.SyncInfo(
                            on_wait=[extra[j]], on_update=[]
                        )
                        insts.insert(pos, nop)
                        pos += 1
                        n_new += 1
                    si.on_wait = keep
                    i = pos
                i += 1
    return n_new


def build_nc(split=True):
    nc = bass.Bass()

    zt_d = nc.declare_dram_parameter("zt", [D, TPC], F32, isOutput=False)
    ztok_d = nc.declare_dram_parameter("ztok", [TPC, D], F32, isOutput=False)
    w_d = nc.declare_dram_parameter("weight", [K, D], F32, isOutput=False)
    idx_d = nc.declare_dram_parameter("idx_out", [P, NT], I32, isOutput=True)
    loss_d = nc.declare_dram_parameter("loss_e", [P, NT], F32, isOutput=True)
    wn_d = nc.declare_dram_parameter("wn_out", [K, D], F32, isOutput=True)

    mult = mybir.AluOpType.mult
    add = mybir.AluOpType.add
    sub = mybir.AluOpType.subtract
    shr = mybir.AluOpType.logical_shift_right
    shl = mybir.AluOpType.logical_shift_left
    COPY = mybir.ActivationFunctionType.Copy
    X = mybir.AxisListType.X

    with TileContext(nc) as tc:
        with (
            tc.tile_pool(name="const", bufs=1) as const,
            tc.tile_pool(name="big", bufs=1) as big,
            tc.tile_pool(name="wp", bufs=1) as wp,
            tc.tile_pool(name="scores", bufs=3) as spool,
            tc.tile_pool(name="small", bufs=3) as small,
            tc.tile_pool(name="dram", bufs=1, space="DRAM") as dram,
            tc.tile_pool(name="mm", bufs=2, space="PSUM") as psum,
        ):
            # ---- constants ----
            iota16 = const.tile([P, NCH], U16, tag="iota16")
            nc.gpsimd.iota(iota16[:], pattern=[[1, NCH]], base=0,
                           channel_multiplier=0)
            ident = const.tile([P, P], F16, tag="ident")
            make_identity(nc, ident[:])

            # ---- persistent tiles ----
            wmm2 = big.tile([P, K], F16, tag="wmm2")
            zmm2 = big.tile([P, TPC], F16, tag="zmm2")
            zh_lo = big.tile([D, TPC], F16, tag="zhlo")
            idx_all = big.tile([P, NT], I32, tag="idx")
            loss_all = big.tile([P, NT], F32, tag="loss")
            zri_all = big.tile([P, NT], F32, tag="zri")

            # ---- z prep ----
            zfull = wp.tile([P, TPC], F32, tag="ztmp")
            nc.sync.dma_start(zfull[0:D, :], zt_d[:, :])
            nc.sync.dma_start(zfull[D:P, :], zt_d[:, :])
            nc.scalar.activation(zh_lo[:], zfull[0:D, :], COPY, scale=SCALE)
            nc.scalar.activation(zmm2[D:P, :], zfull[D:P, :], COPY,
                                 scale=SCALE)
            nc.vector.scalar_tensor_tensor(zmm2[0:D, :], zfull[0:D, :], SCALE,
                                           zh_lo[:], op0=mult, op1=sub)

            # token norms, batched: ztok rows (t*128+p) -> [p, t, d]
            ztok_b = wp.tile([P, NT * D], F32, tag="ztmp")
            nc.sync.dma_start(
                ztok_b[:].rearrange("p (t d) -> p t d", d=D),
                ztok_d.rearrange("(t p) d -> p t d", p=P),
            )
            zsq = wp.tile([P, NT * D], F32, tag="scratch16")
            nc.vector.tensor_mul(zsq[:], ztok_b[:], ztok_b[:])
            zn2 = small.tile([P, NT], F32, tag="zn2")
            nc.vector.reduce_sum(
                zn2[:], zsq[:].rearrange("p (t d) -> p t d", d=D), axis=X
            )
            znr = small.tile([P, NT], F32, tag="znr")
            nc.scalar.sqrt(znr[:], zn2[:])
            nc.vector.reciprocal(zri_all[:], znr[:])

            # ---- codebook prep (batched): rows (p*64+t) -> [p, t, d] ----
            w_big = wp.tile([P, 64 * D], F32, tag="wbig")
            nc.sync.dma_start(
                w_big[:], w_d.rearrange("(p t) d -> p (t d)", p=P)
            )
            wsq = wp.tile([P, 64 * D], F32, tag="scratch16")
            nc.vector.tensor_mul(wsq[:], w_big[:], w_big[:])
            wn2 = small.tile([P, 64], F32, tag="wn2")
            nc.vector.reduce_sum(
                wn2[:], wsq[:].rearrange("p (t d) -> p t d", d=D), axis=X
            )
            wnr = small.tile([P, 64], F32, tag="wnr")
            nc.scalar.sqrt(wnr[:], wn2[:])
            wri = small.tile([P, 64], F32, tag="wri")
            nc.vector.reciprocal(wri[:], wnr[:])
            nc.vector.tensor_mul(
                w_big[:].rearrange("p (t d) -> p t d", d=D),
                w_big[:].rearrange("p (t d) -> p t d", d=D),
                wri[:].to_broadcast([P, 64, D]),
            )
            nc.sync.dma_start(
                wn_d.rearrange("(p t) d -> p (t d)", p=P), w_big[:]
            )
            whl_big = wp.tile([P, 64 * 2 * D], F16, tag="scratch16")
            whl3 = whl_big[:].rearrange("p (t x d) -> p t x d", x=2, d=D)
            w3 = w_big[:].rearrange("p (t d) -> p t d", d=D)
            nc.scalar.activation(whl3[:, :, 0, :], w3, COPY, scale=SCALE)
            nc.vector.scalar_tensor_tensor(whl3[:, :, 1, :], w3, SCALE,
                                           whl3[:, :, 0, :], op0=mult, op1=sub)
            # PE-transpose the interleaved [wh|wl] blocks: input slice t is
            # [128, 128] (cols 0-63 wh, 64-127 wl of code rows p*64+t);
            # transposed psum partitions 0-63 = wh dims, 64-127 = wl dims.
            # Scatter into wmm2 columns t + 64*p so wmm2 column c == code c.
            for b in range(4):  # 16 transposes per psum tile
                pt = psum.tile([P, 16 * P], F16, tag="mm")
                for j in range(16):
                    t_i = b * 16 + j
                    nc.tensor.transpose(
                        pt[:, j * P : (j + 1) * P],
                        whl_big[:, t_i * P : (t_i + 1) * P],
                        ident[:],
                    )
                nc.vector.tensor_copy(
                    wmm2[:, :].rearrange("e (p t) -> e t p", t=64)[
                        :, b * 16 : (b + 1) * 16, :
                    ],
                    pt[:, :].rearrange("e (j p) -> e j p", p=P),
                )

            # ---- per token tile ----
            for t in range(NT):
                scores = spool.tile([P, K], F32, tag="scores")
                zh_t = zh_lo[:, t * P : (t + 1) * P]
                zs_t = zmm2[:, t * P : (t + 1) * P]
                for q in range(NQ):
                    ps = psum.tile([P, QW], F32, tag="mm")
                    for st, sp, lt, kk in ((True, False, zh_t, D),
                                           (False, True, zs_t, P)):
                        for cc in range(QW // CHW):
                            c = q * (QW // CHW) + cc
                            sl = ps[:, cc * CHW : (cc + 1) * CHW]
                            wsl = slice(c * CHW, (c + 1) * CHW)
                            nc.tensor.matmul(sl, lhsT=lt,
                                             rhs=wmm2[0:kk, wsl],
                                             start=st, stop=sp)
                    # write scores interleaved: column j*16 + c so the
                    # chunk dim is contiguous for the acc reduce
                    nc.scalar.copy(
                        scores[:].rearrange("p (j c) -> p c j", c=NCH)[
                            :, q * 4 : (q + 1) * 4, :
                        ],
                        ps[:].rearrange("p (c j) -> p c j", j=CHW),
                    )

                # column-wise max over the 16 chunks: acc[j] = max_c S[c*512+j]
                acc = small.tile([P, CHW], F32, tag="acc")
                if t in GP_TILES:
                    g1 = wp.tile([P, 4096], F32, tag="scratch16")
                    nc.gpsimd.tensor_max(g1[:], scores[:, :4096],
                                         scores[:, 4096:])
                    nc.vector.reduce_max(
                        acc[:],
                        g1[:].rearrange("p (c j) -> p j c", j=CHW),
                        axis=X,
                    )
                else:
                    nc.vector.reduce_max(
                        acc[:],
                        scores[:].rearrange("p (j c) -> p j c", c=NCH),
                        axis=X,
                    )
                gmax = small.tile([P, 1], F32, tag="gmax")
                nc.vector.reduce_max(gmax[:], acc[:], axis=X)
                in8 = small.tile([P, 8], F32, tag="in8")
                nc.gpsimd.memset(in8[:], -3.0e38)
                nc.vector.tensor_copy(in8[:, 0:1], gmax[:])
                j08 = small.tile([P, 8], U32, tag="j08")
                nc.vector.max_index(j08[:], in8[:], acc[:])
                # chunk recovery: gather S[:, 512c + j0] for all c via the
                # group-wrapped gpsimd gather (idxs[p, c] = 512c + j0[p])
                j16 = small.tile([P, 1], U32, tag="j16")
                nc.vector.tensor_scalar(out=j16[:], in0=j08[:, 0:1],
                                        scalar1=4, scalar2=None, op0=shl)
                j0f = small.tile([P, 1], F32, tag="j0f")
                nc.vector.tensor_copy(j0f[:], j16[:])
                gidx = small.tile([P, NCH], U16, tag="gidx")
                nc.vector.tensor_scalar(
                    out=gidx[:], in0=iota16[:], scalar1=j0f[:],
                    scalar2=None, op0=add,
                )
                g256 = small.tile([P, P * 2], F32, tag="g256")
                nc.gpsimd.indirect_copy(g256[:], scores[:], gidx[:], True)
                k8 = small.tile([P, 8], U32, tag="k8")
                nc.vector.max_index(k8[:], in8[:], g256[:])
                cidx = small.tile([P, 1], U32, tag="cidx")
                nc.vector.tensor_scalar(
                    out=cidx[:], in0=k8[:, 0:1], scalar1=4, scalar2=9,
                    op0=shr, op1=shl,
                )
                idxf = small.tile([P, 1], U32, tag="idxf")
                nc.vector.tensor_add(idxf[:], cidx[:], j08[:, 0:1])
                nc.vector.tensor_copy(idx_all[:, t : t + 1], idxf[:])

                # loss term: e = 2 - (2/65536) * gmax * (1/|zf|)
                cosv = small.tile([P, 1], F32, tag="cosv")
                nc.vector.tensor_mul(cosv[:], gmax[:],
                                     zri_all[:, t : t + 1])
                nc.vector.tensor_scalar(
                    out=loss_all[:, t : t + 1], in0=cosv[:],
                    scalar1=-2.0 * INV_SCALE2, scalar2=2.0,
                    op0=mult, op1=add,
                )

            nc.sync.dma_start(idx_d[:, :], idx_all[:])
            nc.sync.dma_start(loss_d[:, :], loss_all[:])

    if split:
        split_waits(nc)
    return nc


def make_in_maps(z, weight):
    z = np.ascontiguousarray(z, dtype=np.float32)
    w = np.ascontiguousarray(weight, dtype=np.float32)
    zf = np.ascontiguousarray(z.transpose(0, 2, 3, 1).reshape(N, D))
    in_maps = []
    for c in range(N_CORES):
        sh = zf[c * TPC : (c + 1) * TPC]
        in_maps.append(
            {
                "zt": np.ascontiguousarray(sh.T),
                "ztok": np.ascontiguousarray(sh),
                "weight": w,
            }
        )
    return in_maps


def assemble(results):
    idx_parts = []
    loss_sum = 0.0
    for r in results:
        idx_parts.append(np.ascontiguousarray(r["idx_out"].T).reshape(-1))
        loss_sum += r["loss_e"].astype(np.float64).sum()
    idx = np.concatenate(idx_parts).astype(np.int32)
    wn = np.asarray(results[0]["wn_out"], dtype=np.float32)

    z_q_flat = wn[idx]
    z_q_out = np.ascontiguousarray(
        z_q_flat.reshape(B, H, W, C).transpose(0, 3, 1, 2)
    ).astype(np.float32)

    encodings = np.zeros((N, K), dtype=np.float32)
    encodings[np.arange(N), idx] = 1.0

    loss = np.float32(1.01 * loss_sum / (N * D))

    counts = np.bincount(idx, minlength=K).astype(np.float64)
    avg_probs = counts / N
    perplexity = np.float32(np.exp(-np.sum(avg_probs * np.log(avg_probs + 1e-10))))

    return z_q_out, idx, loss, encodings, perplexity


def kernel(z, weight):
    from concourse.bass_utils import run_bass_kernel_spmd

    if "nc" not in _CACHE:
        _CACHE["nc"] = build_nc()
    nc = _CACHE["nc"]
    in_maps = make_in_maps(z, weight)
    res = run_bass_kernel_spmd(nc, in_maps, list(range(N_CORES)))
    return assemble(res.results)
